# revision 17
# baseline (speedup 1.0000x reference)
"""BitLinear (RMSNorm + int8 act quant + ternary weight quant + GEMM) on 8 TRN2 cores.

Sharding: 2 token-groups x 4 dout-groups. Each core:
  - x shard [4096, 2048] (token-parallel)
  - wT shard [2048, 2048] = weight[og*2048:(og+1)*2048, :].T  (host pre-transposed)
  - wsc shard [1024, 2048] = weight[c*1024:(c+1)*1024, :]     (global mean|w| pass)
  - norm_weight folded in only when not all-ones (gw input + extra DVE mult)

v3 pipeline: minimize the pre-first-matmul critical path.
  ~7us   warmup AllReduce triggered with NO input deps (garbage dummy bufs)
         -> the one-time ~50us ncfw arming barrier runs 7-57us
  0-30us wsc read (8MB) -> ACT Abs+accum -> DVE reduce -> pin (GPSIMD DMA)
         -> real AllReduce trigger ~40us; runs right after the warmup op
  ~85us  ws/inv_ws; wt (16MB) streams through a small staging pool (SYNC
         issues), DVE-quantizes oc-major into persistent bf16 wq; matmuls
         start as soon as wq[k][:, oc0] chunks land
  phase1 oc-major matmul sweep over tiles 0..7 (overlaps rest of quantize)
  phase2 tile-major: x-chain lookahead + 64 matmuls + ACT drains per tile
  x tiles 0..11 (stats/quant/DMA-transpose) fill the 0-85us window.
Engine placement: Square+sumsq+drains on ACT, amax + quantize on DVE, out-DMA
issues on GPSIMD (SWDGE), wt stream issues on SYNC, collective bounce DMAs on
GPSIMD (their long waits must not head-block the x/wt queues).
The quantized GEMM is exact: x_q in [-127,127] and w_q in {-1,0,1} are exactly
representable in bf16 and PSUM accumulates in f32.
"""

import sys

if "/opt/trn_rl_repo" not in sys.path:
    sys.path.insert(0, "/opt/trn_rl_repo")

import numpy as np

# ---------------------------------------------------------------- config

N_CORES = 8
TG, OG = 2, 4            # token groups x dout groups
B, S, DIN, DOUT = 4, 2048, 2048, 8192
TOKENS = B * S           # 8192
T_SH = TOKENS // TG      # 4096 tokens per core
O_SH = DOUT // OG        # 2048 dout per core
WSC_ROWS = DOUT // N_CORES  # 1024 rows of w per core for the scale pass

P = 128                  # partitions
EPS_NORM = 1e-6
EPS_SCALE = 1e-8
QB = 127.0
C_MAGIC = 12582912.0     # 1.5 * 2^23 : float32 RNE integer-rounding constant
N_W = float(DOUT * DIN)  # elements of weight for the global mean

N_PRE = 12               # x tiles processed before the weight-quantize block
                         # (must be <= qtbuf bufs: transpose N_PRE-1 must not
                         # wait on a slot freed only by post-quantize matmuls)
N_PH1 = 8                # tiles in the oc-major phase-1 matmul sweep
LOOKAHEAD = 4            # x-chain emission lookahead in phase 2


def build_bass(use_gw=False):
    """Build the per-core SPMD Bass graph."""
    import concourse.bass as bass
    import concourse.bacc as bacc
    import concourse.mybir as mybir
    from concourse import tile

    fp32 = mybir.dt.float32
    bf16 = mybir.dt.bfloat16
    Alu = mybir.AluOpType
    Act = mybir.ActivationFunctionType

    t_tiles = T_SH // P          # 32 token tiles
    k_tiles = DIN // P           # 16 contraction tiles
    oc_sz = 512
    oc_chunks = O_SH // oc_sz    # 4 PSUM output chunks per token tile
    wsc_tiles = WSC_ROWS // P    # 8

    nc = bacc.Bacc("TRN2", target_bir_lowering=False, debug=False,
                   num_devices=N_CORES)

    x_d = nc.dram_tensor("x", [T_SH, DIN], fp32, kind="ExternalInput")
    wt_d = nc.dram_tensor("wt", [DIN, O_SH], fp32, kind="ExternalInput")
    wsc_d = nc.dram_tensor("wsc", [WSC_ROWS, DIN], fp32, kind="ExternalInput")
    if use_gw:
        gw_d = nc.dram_tensor("gw", [P, DIN], fp32, kind="ExternalInput")
    out_d = nc.dram_tensor("out", [T_SH, O_SH], fp32, kind="ExternalOutput")

    # collective bounce buffers (internal DRAM)
    pin_d = nc.dram_tensor("cc_in", [P, 1], fp32)
    pout_d = nc.dram_tensor("cc_out", [P, 1], fp32)
    warm_in_d = nc.dram_tensor("cc_warm_in", [P, 1], fp32)
    warm_out_d = nc.dram_tensor("cc_warm_out", [P, 1], fp32)

    with tile.TileContext(nc) as tc:
        with (
            tc.tile_pool(name="persist", bufs=1) as persist,
            tc.tile_pool(name="xin", bufs=3) as xin_pool,        # [P,DIN] f32
            tc.tile_pool(name="wpass", bufs=2) as wpass_pool,    # [P,DIN] f32
            tc.tile_pool(name="scr", bufs=1) as scr_pool,        # [P,DIN] f32
            tc.tile_pool(name="scrw", bufs=1) as scrw_pool,      # [P,512] f32
            tc.tile_pool(name="qbuf", bufs=3) as q_pool,         # [P,DIN] bf16
            tc.tile_pool(name="qtbuf", bufs=12) as qt_pool,      # [P,16,P] bf16
            tc.tile_pool(name="stage", bufs=5) as stage_pool,    # [P,512] f32
            tc.tile_pool(name="tw", bufs=3) as tw_pool,          # [P,512] f32
            tc.tile_pool(name="obuf", bufs=4) as out_pool,       # [P,512] f32
            tc.tile_pool(name="small", bufs=4) as small,
            tc.tile_pool(name="psum", bufs=5, space="PSUM") as psum_pool,
            tc.tile_pool(name="psums", bufs=1, space="PSUM") as psum_s_pool,
        ):
            # ---- warmup collective with NO input dependency: its trigger
            # fires immediately (~7us) so the ncfw arming barrier runs as
            # early as possible. Data is garbage and unused.
            nc.gpsimd.collective_compute(
                "AllReduce", Alu.add,
                replica_groups=[list(range(N_CORES))],
                ins=[warm_in_d[:]], outs=[warm_out_d[:]],
            )

            # ---------------- persistent tiles
            ones_sb = persist.tile([P, P], fp32)
            nc.gpsimd.memset(ones_sb[:], 1.0)
            epsn = persist.tile([P, 1], fp32)
            nc.gpsimd.memset(epsn[:], EPS_NORM)
            if use_gw:
                gw_sb = persist.tile([P, DIN], fp32)
                nc.scalar.dma_start(gw_sb[:], gw_d[:])
            # per-k quantized transposed weight blocks [d_lo, o]
            wq = [persist.tile([P, O_SH], bf16, name=f"wq{k}")
                  for k in range(k_tiles)]
            # per-token stats, one column per token tile
            sumsq_t = persist.tile([P, t_tiles], fp32)
            amax_t = persist.tile([P, t_tiles], fp32)
            m_t = persist.tile([P, t_tiles], fp32)
            alpha_t = persist.tile([P, t_tiles], fp32)
            xs1_t = persist.tile([P, t_tiles], fp32)
            wacc = persist.tile([P, 4 * wsc_tiles], fp32)

            # ---- pass A: per-core sum |wsc| (8MB; gates the AllReduce).
            # Sub-chunked [P,512] Abs so ACT interleaves with x Squares.
            def emit_wsum_tile(j):
                wtile = wpass_pool.tile([P, DIN], fp32, tag="wp")
                nc.scalar.dma_start(wtile[:], wsc_d[j * P:(j + 1) * P, :])
                for s4 in range(4):
                    scw = scrw_pool.tile([P, 512], fp32, tag="scw")
                    nc.scalar.activation(
                        scw[:], wtile[:, s4 * 512:(s4 + 1) * 512], Act.Abs,
                        accum_out=wacc[:, 4 * j + s4:4 * j + s4 + 1])

            ws = small.tile([P, 1], fp32, name="ws")        # written later
            inv_ws = small.tile([P, 1], fp32, name="invws")

            # ---- per-token x tile chain (stats + quantize + transpose)
            def emit_x_tile(i):
                xt = xin_pool.tile([P, DIN], fp32, tag="xin")
                nc.scalar.dma_start(xt[:], x_d[i * P:(i + 1) * P, :])
                scr = scr_pool.tile([P, DIN], fp32, tag="scr")
                # scr = x^2 (discarded); accum -> sumsq
                nc.scalar.activation(scr[:], xt[:], Act.Square,
                                     accum_out=sumsq_t[:, i:i + 1])
                if use_gw:
                    yt = scr_pool.tile([P, DIN], fp32, tag="yt")
                    nc.vector.tensor_tensor(out=yt[:], in0=xt[:], in1=gw_sb[:],
                                            op=Alu.mult)
                    src = yt
                else:
                    src = xt
                nc.vector.tensor_reduce(
                    out=amax_t[:, i:i + 1], in_=src[:], op=Alu.max,
                    axis=mybir.AxisListType.X, apply_absolute_value=True)
                # per-token scalars on [P, 1]
                sq = small.tile([P, 1], fp32, tag="sq")
                nc.scalar.activation(sq[:], sumsq_t[:, i:i + 1], Act.Sqrt,
                                     scale=1.0 / DIN, bias=epsn[:])
                d1 = small.tile([P, 1], fp32, tag="d1")
                nc.vector.tensor_scalar(out=d1[:], in0=amax_t[:, i:i + 1],
                                        scalar1=1.0 / QB, scalar2=None,
                                        op0=Alu.mult)
                rsq = small.tile([P, 1], fp32, tag="rsq")
                nc.vector.reciprocal(rsq[:], sq[:])
                # f1 = d1 + EPS_SCALE*sq ; m = 1/f1
                f1 = small.tile([P, 1], fp32, tag="f1")
                nc.vector.tensor_scalar(out=f1[:], in0=sq[:],
                                        scalar1=EPS_SCALE, scalar2=d1[:],
                                        op0=Alu.mult, op1=Alu.add)
                nc.vector.reciprocal(m_t[:, i:i + 1], f1[:])
                xs0 = small.tile([P, 1], fp32, tag="xs0")
                nc.vector.tensor_tensor(out=xs0[:], in0=d1[:], in1=rsq[:],
                                        op=Alu.mult)
                # alpha = (xs0 + eps) * w_scale.  ws is only written after the
                # AllReduce, so tiles emitted before it store xs1 = xs0 + eps
                # and get alpha in one fix-up op once ws exists.
                if i < N_PRE:
                    nc.vector.tensor_scalar(out=xs1_t[:, i:i + 1], in0=xs0[:],
                                            scalar1=EPS_SCALE, scalar2=None,
                                            op0=Alu.add)
                else:
                    nc.vector.tensor_scalar(out=alpha_t[:, i:i + 1],
                                            in0=xs0[:],
                                            scalar1=EPS_SCALE, scalar2=ws[:],
                                            op0=Alu.add, op1=Alu.mult)
                # quantize in place: xt = round(src*m) + C via magic constant
                nc.vector.tensor_scalar(out=xt[:], in0=src[:],
                                        scalar1=m_t[:, i:i + 1],
                                        scalar2=C_MAGIC,
                                        op0=Alu.mult, op1=Alu.add)
                qt8 = q_pool.tile([P, DIN], bf16, tag="q")
                nc.vector.tensor_scalar(out=qt8[:], in0=xt[:], scalar1=C_MAGIC,
                                        scalar2=None, op0=Alu.subtract)
                # one xbar transpose for the whole tile:
                # qT[d_lo, k, t] = qt8[t, 128k + d_lo]
                qT = qt_pool.tile([P, k_tiles, P], bf16, tag="qT")
                nc.sync.dma_start(out=qT[:], in_=qt8[:], transpose=True)
                return qT

            qT_tiles = [None] * t_tiles

            # emission: wsc tiles with x0/x1 interleaved
            for j in range(4):
                emit_wsum_tile(j)
            qT_tiles[0] = emit_x_tile(0)
            for j in range(4, wsc_tiles):
                emit_wsum_tile(j)
            qT_tiles[1] = emit_x_tile(1)

            # ---- real AllReduce of per-partition |w| sums.  The pin/pout
            # DMAs live on GPSIMD: they wait on the DVE reduce / the
            # collective, and on the scalar or sync queues that wait would
            # head-block the x-tile loads or the wt stream.
            wpart = small.tile([P, 1], fp32)
            nc.vector.tensor_reduce(out=wpart[:], in_=wacc[:], op=Alu.add,
                                    axis=mybir.AxisListType.X)
            nc.gpsimd.dma_start(pin_d[:], wpart[:])
            nc.gpsimd.collective_compute(
                "AllReduce", Alu.add,
                replica_groups=[list(range(N_CORES))],
                ins=[pin_d[:]], outs=[pout_d[:]],
            )
            wsum_all = small.tile([P, 1], fp32)
            nc.gpsimd.dma_start(wsum_all[:], pout_d[:])

            # x tiles 2..N_PRE-1 while the collective runs
            for i in range(2, N_PRE):
                qT_tiles[i] = emit_x_tile(i)

            # cross-partition sum + broadcast via ones matmul
            psum_s = psum_s_pool.tile([P, 512], fp32, tag="pss")
            nc.tensor.matmul(psum_s[:, 0:1], ones_sb[:], wsum_all[:],
                             start=True, stop=True)
            ssum = small.tile([P, 1], fp32)
            nc.vector.tensor_copy(ssum[:], psum_s[:, 0:1])
            nc.vector.tensor_scalar(out=ws[:], in0=ssum[:],
                                    scalar1=1.0 / N_W,
                                    scalar2=EPS_SCALE,
                                    op0=Alu.mult, op1=Alu.add)
            nc.vector.reciprocal(inv_ws[:], ws[:])
            # deferred alpha for the pre-AllReduce tiles
            nc.vector.tensor_scalar(out=alpha_t[:, 0:N_PRE],
                                    in0=xs1_t[:, 0:N_PRE],
                                    scalar1=ws[:], scalar2=None, op0=Alu.mult)

            # ---- wt read (16MB) + quantize -> ternary bf16, oc-major so the
            # first oc chunk of all k tiles completes first.
            for oc in range(oc_chunks):
                osl = slice(oc * oc_sz, (oc + 1) * oc_sz)
                for k in range(k_tiles):
                    st = stage_pool.tile([P, oc_sz], fp32, tag="st")
                    nc.sync.dma_start(st[:], wt_d[k * P:(k + 1) * P, osl])
                    tw1 = tw_pool.tile([P, oc_sz], fp32, tag="tw1")
                    nc.vector.tensor_scalar(out=tw1[:], in0=st[:],
                                            scalar1=inv_ws[:],
                                            scalar2=C_MAGIC,
                                            op0=Alu.mult, op1=Alu.add)
                    tw2 = tw_pool.tile([P, oc_sz], fp32, tag="tw2")
                    # clip in C-space: exact for |q| <= ~2^22
                    nc.vector.tensor_scalar(out=tw2[:], in0=tw1[:],
                                            scalar1=C_MAGIC + 1.0,
                                            scalar2=C_MAGIC - 1.0,
                                            op0=Alu.min, op1=Alu.max)
                    nc.vector.tensor_scalar(out=wq[k][:, osl], in0=tw2[:],
                                            scalar1=C_MAGIC, scalar2=None,
                                            op0=Alu.subtract)

            # ---- matmul chain for one (tile, oc) pair
            def emit_mm_chain(i, oc):
                pt = psum_pool.tile([P, oc_sz], fp32, tag="ps")
                qT = qT_tiles[i]
                for k in range(k_tiles):
                    nc.tensor.matmul(pt[:], qT[:, k, :],
                                     wq[k][:, oc * oc_sz:(oc + 1) * oc_sz],
                                     start=(k == 0), stop=(k == k_tiles - 1))
                osb = out_pool.tile([P, oc_sz], fp32, tag="o")
                nc.scalar.activation(osb[:], pt[:], Act.Copy,
                                     scale=alpha_t[:, i:i + 1])
                nc.gpsimd.dma_start(
                    out_d[i * P:(i + 1) * P, oc * oc_sz:(oc + 1) * oc_sz],
                    osb[:])

            # phase 1: oc-major across tiles 0..N_PH1-1 (paced by quantize)
            for oc in range(oc_chunks):
                for i in range(N_PH1):
                    emit_mm_chain(i, oc)

            # phase 2: tile-major with x-chain lookahead
            for i in range(N_PRE, N_PH1 + LOOKAHEAD):
                qT_tiles[i] = emit_x_tile(i)
            for i in range(N_PH1, t_tiles):
                j = i + LOOKAHEAD
                if max(N_PRE, N_PH1 + LOOKAHEAD) <= j < t_tiles:
                    qT_tiles[j] = emit_x_tile(j)
                for oc in range(oc_chunks):
                    emit_mm_chain(i, oc)

    nc.compile()
    return nc


# ---------------------------------------------------------------- host wrapper

_CACHED = {}


def _get_nc(use_gw):
    key = ("nc", use_gw)
    if key not in _CACHED:
        _CACHED[key] = build_bass(use_gw=use_gw)
    return _CACHED[key]


def kernel(x: np.ndarray, weight: np.ndarray, norm_weight: np.ndarray) -> np.ndarray:
    from concourse.bass_utils import run_bass_kernel_spmd

    assert x.shape == (B, S, DIN) and weight.shape == (DOUT, DIN)
    use_gw = not bool(np.all(norm_weight == 1.0))
    x_flat = np.ascontiguousarray(x.reshape(TOKENS, DIN), dtype=np.float32)
    w = np.ascontiguousarray(weight, dtype=np.float32)
    wt_full = np.ascontiguousarray(w.T)  # [DIN, DOUT]

    in_maps = []
    for c in range(N_CORES):
        tg, og = divmod(c, OG)
        im = {
            "x": np.ascontiguousarray(x_flat[tg * T_SH:(tg + 1) * T_SH]),
            "wt": np.ascontiguousarray(wt_full[:, og * O_SH:(og + 1) * O_SH]),
            "wsc": np.ascontiguousarray(w[c * WSC_ROWS:(c + 1) * WSC_ROWS]),
        }
        if use_gw:
            im["gw"] = np.ascontiguousarray(
                np.broadcast_to(norm_weight.astype(np.float32), (P, DIN)))
        in_maps.append(im)

    nc = _get_nc(use_gw)
    res = run_bass_kernel_spmd(nc, in_maps, core_ids=list(range(N_CORES)))
    _CACHED["last_results"] = res

    out = np.empty((TOKENS, DOUT), dtype=np.float32)
    for c in range(N_CORES):
        tg, og = divmod(c, OG)
        out[tg * T_SH:(tg + 1) * T_SH, og * O_SH:(og + 1) * O_SH] = \
            res.results[c]["out"]
    return out.reshape(B, S, DOUT)


# revision 26
# speedup vs baseline: 1.0319x; 1.0319x over previous
"""BitLinear (RMSNorm + int8 act quant + ternary weight quant + GEMM) on 8 TRN2 cores.

Sharding: 2 token-groups x 4 dout-groups. Each core:
  - x shard [4096, 2048] (token-parallel)
  - wT shard [2048, 2048] = weight[og*2048:(og+1)*2048, :].T  (host pre-transposed)
  - wsc shard [1024, 2048] = weight[c*1024:(c+1)*1024, :]     (global mean|w| pass)
  - norm_weight folded in only when not all-ones (gw input + extra DVE mult)

v3 pipeline: minimize the pre-first-matmul critical path.
  ~7us   warmup AllReduce triggered with NO input deps (garbage dummy bufs)
         -> the one-time ~50us ncfw arming barrier runs 7-57us
  0-30us wsc read (8MB) -> ACT Abs+accum -> DVE reduce -> pin (GPSIMD DMA)
         -> real AllReduce trigger ~40us; runs right after the warmup op
  ~85us  ws/inv_ws; wt (16MB) streams through a small staging pool (SYNC
         issues), DVE-quantizes oc-major into persistent bf16 wq; matmuls
         start as soon as wq[k][:, oc0] chunks land
  phase1 oc-major matmul sweep over tiles 0..7 (overlaps rest of quantize)
  phase2 tile-major: x-chain lookahead + 64 matmuls + ACT drains per tile
  x tiles 0..11 (stats/quant/DMA-transpose) fill the 0-85us window.
Engine placement: Square+sumsq+drains on ACT, amax + quantize on DVE, out-DMA
issues on GPSIMD (SWDGE), wt stream issues on SYNC, collective bounce DMAs on
GPSIMD (their long waits must not head-block the x/wt queues).
The quantized GEMM is exact: x_q in [-127,127] and w_q in {-1,0,1} are exactly
representable in bf16 and PSUM accumulates in f32.
"""

import sys

if "/opt/trn_rl_repo" not in sys.path:
    sys.path.insert(0, "/opt/trn_rl_repo")

import numpy as np

# ---------------------------------------------------------------- config

N_CORES = 8
TG, OG = 2, 4            # token groups x dout groups
B, S, DIN, DOUT = 4, 2048, 2048, 8192
TOKENS = B * S           # 8192
T_SH = TOKENS // TG      # 4096 tokens per core
O_SH = DOUT // OG        # 2048 dout per core
WSC_ROWS = DOUT // N_CORES  # 1024 rows of w per core for the scale pass

P = 128                  # partitions
EPS_NORM = 1e-6
EPS_SCALE = 1e-8
QB = 127.0
C_MAGIC = 12582912.0     # 1.5 * 2^23 : float32 RNE integer-rounding constant
N_W = float(DOUT * DIN)  # elements of weight for the global mean

N_PRE = 12               # x tiles processed before the weight-quantize block
                         # (must be <= qtbuf bufs: transpose N_PRE-1 must not
                         # wait on a slot freed only by post-quantize matmuls)
N_PH1 = 8                # tiles in the oc-major phase-1 matmul sweep
LOOKAHEAD = 4            # x-chain emission lookahead in phase 2


def build_bass(use_gw=False):
    """Build the per-core SPMD Bass graph."""
    import concourse.bass as bass
    import concourse.bacc as bacc
    import concourse.mybir as mybir
    from concourse import tile

    fp32 = mybir.dt.float32
    bf16 = mybir.dt.bfloat16
    Alu = mybir.AluOpType
    Act = mybir.ActivationFunctionType

    t_tiles = T_SH // P          # 32 token tiles
    k_tiles = DIN // P           # 16 contraction tiles
    oc_sz = 512
    oc_chunks = O_SH // oc_sz    # 4 PSUM output chunks per token tile
    wsc_tiles = WSC_ROWS // P    # 8

    nc = bacc.Bacc("TRN2", target_bir_lowering=False, debug=False,
                   num_devices=N_CORES)

    x_d = nc.dram_tensor("x", [T_SH, DIN], fp32, kind="ExternalInput")
    wt_d = nc.dram_tensor("wt", [DIN, O_SH], fp32, kind="ExternalInput")
    wsc_d = nc.dram_tensor("wsc", [WSC_ROWS, DIN], fp32, kind="ExternalInput")
    if use_gw:
        gw_d = nc.dram_tensor("gw", [P, DIN], fp32, kind="ExternalInput")
    out_d = nc.dram_tensor("out", [T_SH, O_SH], fp32, kind="ExternalOutput")

    # collective bounce buffers (internal DRAM)
    pin_d = nc.dram_tensor("cc_in", [P, 1], fp32)
    pout_d = nc.dram_tensor("cc_out", [P, 1], fp32)
    warm_in_d = nc.dram_tensor("cc_warm_in", [P, 1], fp32)
    warm_out_d = nc.dram_tensor("cc_warm_out", [P, 1], fp32)

    with tile.TileContext(nc) as tc:
        with (
            tc.tile_pool(name="persist", bufs=1) as persist,
            tc.tile_pool(name="xin", bufs=3) as xin_pool,        # [P,DIN] f32
            tc.tile_pool(name="wpass", bufs=2) as wpass_pool,    # [P,DIN] f32
            tc.tile_pool(name="scr", bufs=1) as scr_pool,        # [P,DIN] f32
            tc.tile_pool(name="scrw", bufs=1) as scrw_pool,      # [P,512] f32
            tc.tile_pool(name="qbuf", bufs=2) as q_pool,         # [P,DIN] bf16
            tc.tile_pool(name="qtbuf", bufs=12) as qt_pool,      # [P,16,P] bf16
            tc.tile_pool(name="stage", bufs=3) as stage_pool,    # [P,512] f32
            tc.tile_pool(name="tw", bufs=2) as tw_pool,          # [P,512] f32
            tc.tile_pool(name="obuf", bufs=2) as out_pool,       # [P,DIN] f32
            tc.tile_pool(name="obufc", bufs=2) as outc_pool,     # [P,512] f32
            tc.tile_pool(name="small", bufs=4) as small,
            tc.tile_pool(name="psum", bufs=5, space="PSUM") as psum_pool,
            tc.tile_pool(name="psums", bufs=1, space="PSUM") as psum_s_pool,
        ):
            # ---- warmup collective with NO input dependency: its trigger
            # fires immediately (~7us) so the ncfw arming barrier runs as
            # early as possible. Data is garbage and unused.
            nc.gpsimd.collective_compute(
                "AllReduce", Alu.add,
                replica_groups=[list(range(N_CORES))],
                ins=[warm_in_d[:]], outs=[warm_out_d[:]],
            )

            # ---------------- persistent tiles
            ones_sb = persist.tile([P, P], fp32)
            nc.gpsimd.memset(ones_sb[:], 1.0)
            epsn = persist.tile([P, 1], fp32)
            nc.gpsimd.memset(epsn[:], EPS_NORM)
            if use_gw:
                gw_sb = persist.tile([P, DIN], fp32)
                nc.scalar.dma_start(gw_sb[:], gw_d[:])
            # per-k quantized transposed weight blocks [d_lo, o]
            wq = [persist.tile([P, O_SH], bf16, name=f"wq{k}")
                  for k in range(k_tiles)]
            # per-token stats, one column per token tile
            sumsq_t = persist.tile([P, t_tiles], fp32)
            amax_t = persist.tile([P, t_tiles], fp32)
            m_t = persist.tile([P, t_tiles], fp32)
            alpha_t = persist.tile([P, t_tiles], fp32)
            xs1_t = persist.tile([P, t_tiles], fp32)
            wacc = persist.tile([P, 4 * wsc_tiles], fp32)

            # ---- pass A: per-core sum |wsc| (8MB; gates the AllReduce).
            # Sub-chunked [P,512] Abs so ACT interleaves with x Squares.
            def emit_wsum_tile(j):
                wtile = wpass_pool.tile([P, DIN], fp32, tag="wp")
                nc.scalar.dma_start(wtile[:], wsc_d[j * P:(j + 1) * P, :])
                for s4 in range(4):
                    scw = scrw_pool.tile([P, 512], fp32, tag="scw")
                    nc.scalar.activation(
                        scw[:], wtile[:, s4 * 512:(s4 + 1) * 512], Act.Abs,
                        accum_out=wacc[:, 4 * j + s4:4 * j + s4 + 1])

            ws = small.tile([P, 1], fp32, name="ws")        # written later
            inv_ws = small.tile([P, 1], fp32, name="invws")

            # ---- per-token x tile chain (stats + quantize + transpose)
            def emit_x_tile(i):
                xt = xin_pool.tile([P, DIN], fp32, tag="xin")
                nc.scalar.dma_start(xt[:], x_d[i * P:(i + 1) * P, :])
                scr = scr_pool.tile([P, DIN], fp32, tag="scr")
                # scr = x^2 (discarded); accum -> sumsq
                nc.scalar.activation(scr[:], xt[:], Act.Square,
                                     accum_out=sumsq_t[:, i:i + 1])
                if use_gw:
                    yt = scr_pool.tile([P, DIN], fp32, tag="yt")
                    nc.vector.tensor_tensor(out=yt[:], in0=xt[:], in1=gw_sb[:],
                                            op=Alu.mult)
                    src = yt
                else:
                    src = xt
                nc.vector.tensor_reduce(
                    out=amax_t[:, i:i + 1], in_=src[:], op=Alu.max,
                    axis=mybir.AxisListType.X, apply_absolute_value=True)
                # per-token scalars on [P, 1]
                sq = small.tile([P, 1], fp32, tag="sq")
                nc.scalar.activation(sq[:], sumsq_t[:, i:i + 1], Act.Sqrt,
                                     scale=1.0 / DIN, bias=epsn[:])
                d1 = small.tile([P, 1], fp32, tag="d1")
                nc.vector.tensor_scalar(out=d1[:], in0=amax_t[:, i:i + 1],
                                        scalar1=1.0 / QB, scalar2=None,
                                        op0=Alu.mult)
                rsq = small.tile([P, 1], fp32, tag="rsq")
                nc.vector.reciprocal(rsq[:], sq[:])
                # f1 = d1 + EPS_SCALE*sq ; m = 1/f1
                f1 = small.tile([P, 1], fp32, tag="f1")
                nc.vector.tensor_scalar(out=f1[:], in0=sq[:],
                                        scalar1=EPS_SCALE, scalar2=d1[:],
                                        op0=Alu.mult, op1=Alu.add)
                nc.vector.reciprocal(m_t[:, i:i + 1], f1[:])
                xs0 = small.tile([P, 1], fp32, tag="xs0")
                nc.vector.tensor_tensor(out=xs0[:], in0=d1[:], in1=rsq[:],
                                        op=Alu.mult)
                # alpha = (xs0 + eps) * w_scale.  ws is only written after the
                # AllReduce, so tiles emitted before it store xs1 = xs0 + eps
                # and get alpha in one fix-up op once ws exists.
                if i < N_PRE:
                    nc.vector.tensor_scalar(out=xs1_t[:, i:i + 1], in0=xs0[:],
                                            scalar1=EPS_SCALE, scalar2=None,
                                            op0=Alu.add)
                else:
                    nc.vector.tensor_scalar(out=alpha_t[:, i:i + 1],
                                            in0=xs0[:],
                                            scalar1=EPS_SCALE, scalar2=ws[:],
                                            op0=Alu.add, op1=Alu.mult)
                # quantize in place: xt = round(src*m) + C via magic constant
                nc.vector.tensor_scalar(out=xt[:], in0=src[:],
                                        scalar1=m_t[:, i:i + 1],
                                        scalar2=C_MAGIC,
                                        op0=Alu.mult, op1=Alu.add)
                qt8 = q_pool.tile([P, DIN], bf16, tag="q")
                nc.vector.tensor_scalar(out=qt8[:], in0=xt[:], scalar1=C_MAGIC,
                                        scalar2=None, op0=Alu.subtract)
                # one xbar transpose for the whole tile:
                # qT[d_lo, k, t] = qt8[t, 128k + d_lo]
                qT = qt_pool.tile([P, k_tiles, P], bf16, tag="qT")
                nc.sync.dma_start(out=qT[:], in_=qt8[:], transpose=True)
                return qT

            qT_tiles = [None] * t_tiles

            # emission: wsc tiles with x0/x1 interleaved
            for j in range(4):
                emit_wsum_tile(j)
            qT_tiles[0] = emit_x_tile(0)
            for j in range(4, wsc_tiles):
                emit_wsum_tile(j)
            qT_tiles[1] = emit_x_tile(1)

            # ---- real AllReduce of per-partition |w| sums.  The pin/pout
            # DMAs live on GPSIMD: they wait on the DVE reduce / the
            # collective, and on the scalar or sync queues that wait would
            # head-block the x-tile loads or the wt stream.
            wpart = small.tile([P, 1], fp32)
            with tc.high_priority():
                nc.vector.tensor_reduce(out=wpart[:], in_=wacc[:], op=Alu.add,
                                        axis=mybir.AxisListType.X)
                nc.gpsimd.dma_start(pin_d[:], wpart[:])
                nc.gpsimd.collective_compute(
                    "AllReduce", Alu.add,
                    replica_groups=[list(range(N_CORES))],
                    ins=[pin_d[:]], outs=[pout_d[:]],
                )
                wsum_all = small.tile([P, 1], fp32)
                nc.gpsimd.dma_start(wsum_all[:], pout_d[:])

            # x tiles 2..N_PRE-1 while the collective runs
            for i in range(2, N_PRE):
                qT_tiles[i] = emit_x_tile(i)

            # cross-partition sum + broadcast via ones matmul
            psum_s = psum_s_pool.tile([P, 512], fp32, tag="pss")
            nc.tensor.matmul(psum_s[:, 0:1], ones_sb[:], wsum_all[:],
                             start=True, stop=True)
            ssum = small.tile([P, 1], fp32)
            nc.vector.tensor_copy(ssum[:], psum_s[:, 0:1])
            nc.vector.tensor_scalar(out=ws[:], in0=ssum[:],
                                    scalar1=1.0 / N_W,
                                    scalar2=EPS_SCALE,
                                    op0=Alu.mult, op1=Alu.add)
            nc.vector.reciprocal(inv_ws[:], ws[:])
            # deferred alpha for the pre-AllReduce tiles
            nc.vector.tensor_scalar(out=alpha_t[:, 0:N_PRE],
                                    in0=xs1_t[:, 0:N_PRE],
                                    scalar1=ws[:], scalar2=None, op0=Alu.mult)

            # ---- wt read (16MB) + quantize -> ternary bf16, oc-major so the
            # first oc chunk of all k tiles completes first.
            for oc in range(oc_chunks):
                osl = slice(oc * oc_sz, (oc + 1) * oc_sz)
                for k in range(k_tiles):
                    st = stage_pool.tile([P, oc_sz], fp32, tag="st")
                    nc.sync.dma_start(st[:], wt_d[k * P:(k + 1) * P, osl])
                    tw1 = tw_pool.tile([P, oc_sz], fp32, tag="tw1")
                    nc.vector.tensor_scalar(out=tw1[:], in0=st[:],
                                            scalar1=inv_ws[:],
                                            scalar2=C_MAGIC,
                                            op0=Alu.mult, op1=Alu.add)
                    tw2 = tw_pool.tile([P, oc_sz], fp32, tag="tw2")
                    # clip in C-space: exact for |q| <= ~2^22
                    nc.vector.tensor_scalar(out=tw2[:], in0=tw1[:],
                                            scalar1=C_MAGIC + 1.0,
                                            scalar2=C_MAGIC - 1.0,
                                            op0=Alu.min, op1=Alu.max)
                    nc.vector.tensor_scalar(out=wq[k][:, osl], in0=tw2[:],
                                            scalar1=C_MAGIC, scalar2=None,
                                            op0=Alu.subtract)

            # ---- matmul chains for one tile: 4 PSUM chunks, ACT drains into
            # one [P, O_SH] staging tile, a single per-tile out DMA on SYNC.
            osb_tiles = {}

            def emit_mm_chain(i, oc, oc_major=False):
                pt = psum_pool.tile([P, oc_sz], fp32, tag="ps")
                qT = qT_tiles[i]
                for k in range(k_tiles):
                    nc.tensor.matmul(pt[:], qT[:, k, :],
                                     wq[k][:, oc * oc_sz:(oc + 1) * oc_sz],
                                     start=(k == 0), stop=(k == k_tiles - 1))
                if oc_major:
                    # phase 1 walks oc-major across tiles: per-chunk staging
                    # + per-chunk out DMA (a per-tile osb would pin N_PH1
                    # buffers at once)
                    osb = outc_pool.tile([P, oc_sz], fp32, tag="oc")
                    nc.scalar.activation(osb[:], pt[:], Act.Copy,
                                         scale=alpha_t[:, i:i + 1])
                    nc.sync.dma_start(
                        out_d[i * P:(i + 1) * P,
                              oc * oc_sz:(oc + 1) * oc_sz], osb[:])
                    return
                if oc == 0:
                    osb_tiles[i] = out_pool.tile([P, O_SH], fp32, tag="o",
                                                 name="osb")
                osb = osb_tiles[i]
                nc.scalar.activation(osb[:, oc * oc_sz:(oc + 1) * oc_sz],
                                     pt[:], Act.Copy,
                                     scale=alpha_t[:, i:i + 1])
                if oc == oc_chunks - 1:
                    nc.sync.dma_start(out_d[i * P:(i + 1) * P, :], osb[:])

            # phase 1: oc-major across tiles 0..N_PH1-1 (paced by quantize).
            # tile-major within the last oc pass so each tile's out DMA can
            # issue as soon as its four chunks are drained.
            for oc in range(oc_chunks):
                for i in range(N_PH1):
                    emit_mm_chain(i, oc, oc_major=True)

            # phase 2: tile-major with x-chain lookahead
            for i in range(N_PRE, N_PH1 + LOOKAHEAD):
                qT_tiles[i] = emit_x_tile(i)
            for i in range(N_PH1, t_tiles):
                j = i + LOOKAHEAD
                if max(N_PRE, N_PH1 + LOOKAHEAD) <= j < t_tiles:
                    qT_tiles[j] = emit_x_tile(j)
                for oc in range(oc_chunks):
                    emit_mm_chain(i, oc)

    nc.compile()
    return nc


# ---------------------------------------------------------------- host wrapper

_CACHED = {}


def _get_nc(use_gw):
    key = ("nc", use_gw)
    if key not in _CACHED:
        _CACHED[key] = build_bass(use_gw=use_gw)
    return _CACHED[key]


def kernel(x: np.ndarray, weight: np.ndarray, norm_weight: np.ndarray) -> np.ndarray:
    from concourse.bass_utils import run_bass_kernel_spmd

    assert x.shape == (B, S, DIN) and weight.shape == (DOUT, DIN)
    use_gw = not bool(np.all(norm_weight == 1.0))
    x_flat = np.ascontiguousarray(x.reshape(TOKENS, DIN), dtype=np.float32)
    w = np.ascontiguousarray(weight, dtype=np.float32)
    wt_full = np.ascontiguousarray(w.T)  # [DIN, DOUT]

    in_maps = []
    for c in range(N_CORES):
        tg, og = divmod(c, OG)
        im = {
            "x": np.ascontiguousarray(x_flat[tg * T_SH:(tg + 1) * T_SH]),
            "wt": np.ascontiguousarray(wt_full[:, og * O_SH:(og + 1) * O_SH]),
            "wsc": np.ascontiguousarray(w[c * WSC_ROWS:(c + 1) * WSC_ROWS]),
        }
        if use_gw:
            im["gw"] = np.ascontiguousarray(
                np.broadcast_to(norm_weight.astype(np.float32), (P, DIN)))
        in_maps.append(im)

    nc = _get_nc(use_gw)
    res = run_bass_kernel_spmd(nc, in_maps, core_ids=list(range(N_CORES)))
    _CACHED["last_results"] = res

    out = np.empty((TOKENS, DOUT), dtype=np.float32)
    for c in range(N_CORES):
        tg, og = divmod(c, OG)
        out[tg * T_SH:(tg + 1) * T_SH, og * O_SH:(og + 1) * O_SH] = \
            res.results[c]["out"]
    return out.reshape(B, S, DOUT)


# revision 29
# speedup vs baseline: 1.0516x; 1.0190x over previous
"""BitLinear (RMSNorm + int8 act quant + ternary weight quant + GEMM) on 8 TRN2 cores.

Sharding: 2 token-groups x 4 dout-groups. Each core:
  - x shard [4096, 2048] (token-parallel)
  - wT shard [2048, 2048] = weight[og*2048:(og+1)*2048, :].T  (host pre-transposed)
  - wsc shard [1024, 2048] = weight[c*1024:(c+1)*1024, :]     (global mean|w| pass)
  - norm_weight folded in only when not all-ones (gw input + extra DVE mult)

v3 pipeline: minimize the pre-first-matmul critical path.
  ~7us   warmup AllReduce triggered with NO input deps (garbage dummy bufs)
         -> the one-time ~50us ncfw arming barrier runs 7-57us
  0-30us wsc read (8MB) -> ACT Abs+accum -> DVE reduce -> pin (GPSIMD DMA)
         -> real AllReduce trigger ~40us; runs right after the warmup op
  ~85us  ws/inv_ws; wt (16MB) streams through a small staging pool (SYNC
         issues), DVE-quantizes oc-major into persistent bf16 wq; matmuls
         start as soon as wq[k][:, oc0] chunks land
  phase1 oc-major matmul sweep over tiles 0..7 (overlaps rest of quantize)
  phase2 tile-major: x-chain lookahead + 64 matmuls + ACT drains per tile
  x tiles 0..11 (stats/quant/DMA-transpose) fill the 0-85us window.
Engine placement: Square+sumsq+drains on ACT, amax + quantize on DVE, out-DMA
issues on GPSIMD (SWDGE), wt stream issues on SYNC, collective bounce DMAs on
GPSIMD (their long waits must not head-block the x/wt queues).
The quantized GEMM is exact: x_q in [-127,127] and w_q in {-1,0,1} are exactly
representable in bf16 and PSUM accumulates in f32.
"""

import sys

if "/opt/trn_rl_repo" not in sys.path:
    sys.path.insert(0, "/opt/trn_rl_repo")

import numpy as np

# ---------------------------------------------------------------- config

N_CORES = 8
TG, OG = 2, 4            # token groups x dout groups
B, S, DIN, DOUT = 4, 2048, 2048, 8192
TOKENS = B * S           # 8192
T_SH = TOKENS // TG      # 4096 tokens per core
O_SH = DOUT // OG        # 2048 dout per core
WSC_ROWS = DOUT // N_CORES  # 1024 rows of w per core for the scale pass

P = 128                  # partitions
EPS_NORM = 1e-6
EPS_SCALE = 1e-8
QB = 127.0
C_MAGIC = 12582912.0     # 1.5 * 2^23 : float32 RNE integer-rounding constant
N_W = float(DOUT * DIN)  # elements of weight for the global mean

N_PRE = 12               # x tiles processed before the weight-quantize block
                         # (must be <= qtbuf bufs: transpose N_PRE-1 must not
                         # wait on a slot freed only by post-quantize matmuls)
N_PH1 = 8                # tiles in the oc-major phase-1 matmul sweep
LOOKAHEAD = 4            # x-chain emission lookahead in phase 2


def build_bass(use_gw=False):
    """Build the per-core SPMD Bass graph."""
    import concourse.bass as bass
    import concourse.bacc as bacc
    import concourse.mybir as mybir
    from concourse import tile

    fp32 = mybir.dt.float32
    bf16 = mybir.dt.bfloat16
    Alu = mybir.AluOpType
    Act = mybir.ActivationFunctionType

    t_tiles = T_SH // P          # 32 token tiles
    k_tiles = DIN // P           # 16 contraction tiles
    oc_sz = 512
    oc_chunks = O_SH // oc_sz    # 4 PSUM output chunks per token tile
    wsc_tiles = WSC_ROWS // P    # 8

    nc = bacc.Bacc("TRN2", target_bir_lowering=False, debug=False,
                   num_devices=N_CORES)

    x_d = nc.dram_tensor("x", [T_SH, DIN], fp32, kind="ExternalInput")
    wt_d = nc.dram_tensor("wt", [DIN, O_SH], fp32, kind="ExternalInput")
    wsc_d = nc.dram_tensor("wsc", [WSC_ROWS, DIN], fp32, kind="ExternalInput")
    if use_gw:
        gw_d = nc.dram_tensor("gw", [P, DIN], fp32, kind="ExternalInput")
    out_d = nc.dram_tensor("out", [T_SH, O_SH], fp32, kind="ExternalOutput")

    # collective bounce buffers (internal DRAM)
    pin_d = nc.dram_tensor("cc_in", [P, 1], fp32)
    pout_d = nc.dram_tensor("cc_out", [P, 1], fp32)
    warm_in_d = nc.dram_tensor("cc_warm_in", [P, 1], fp32)
    warm_out_d = nc.dram_tensor("cc_warm_out", [P, 1], fp32)

    with tile.TileContext(nc) as tc:
        with (
            tc.tile_pool(name="persist", bufs=1) as persist,
            tc.tile_pool(name="xin", bufs=3) as xin_pool,        # [P,DIN] f32
            tc.tile_pool(name="wpass", bufs=2) as wpass_pool,    # [P,DIN] f32
            tc.tile_pool(name="scr", bufs=1) as scr_pool,        # [P,DIN] f32
            tc.tile_pool(name="scrw", bufs=1) as scrw_pool,      # [P,512] f32
            tc.tile_pool(name="qbuf", bufs=2) as q_pool,         # [P,DIN] bf16
            tc.tile_pool(name="qtbuf", bufs=12) as qt_pool,      # [P,16,P] bf16
            tc.tile_pool(name="stage", bufs=3) as stage_pool,    # [P,512] f32
            tc.tile_pool(name="tw", bufs=2) as tw_pool,          # [P,512] f32
            tc.tile_pool(name="obuf", bufs=2) as out_pool,       # [P,DIN] f32
            tc.tile_pool(name="obufc", bufs=2) as outc_pool,     # [P,512] f32
            tc.tile_pool(name="small", bufs=4) as small,
            tc.tile_pool(name="psum", bufs=5, space="PSUM") as psum_pool,
            tc.tile_pool(name="psums", bufs=1, space="PSUM") as psum_s_pool,
        ):
            # ---- warmup collective with NO input dependency: its trigger
            # fires immediately (~7us) so the ncfw arming barrier runs as
            # early as possible. Data is garbage and unused.
            nc.gpsimd.collective_compute(
                "AllReduce", Alu.add,
                replica_groups=[list(range(N_CORES))],
                ins=[warm_in_d[:]], outs=[warm_out_d[:]],
            )

            # ---------------- persistent tiles
            ones_sb = persist.tile([P, P], fp32)
            nc.gpsimd.memset(ones_sb[:], 1.0)
            epsn = persist.tile([P, 1], fp32)
            nc.gpsimd.memset(epsn[:], EPS_NORM)
            if use_gw:
                gw_sb = persist.tile([P, DIN], fp32)
                nc.scalar.dma_start(gw_sb[:], gw_d[:])
            # per-k quantized transposed weight blocks [d_lo, o]
            wq = [persist.tile([P, O_SH], bf16, name=f"wq{k}")
                  for k in range(k_tiles)]
            # per-token stats, one column per token tile
            sumsq_t = persist.tile([P, t_tiles], fp32)
            amax_t = persist.tile([P, t_tiles], fp32)
            m_t = persist.tile([P, t_tiles], fp32)
            alpha_t = persist.tile([P, t_tiles], fp32)
            xs1_t = persist.tile([P, t_tiles], fp32)
            wacc = persist.tile([P, 4 * wsc_tiles], fp32)

            # ---- pass A: per-core sum |wsc| (8MB; gates the AllReduce).
            # High priority + the SYNC DMA ring: the scheduler must not
            # queue these loads behind the x-tile loads (priority does not
            # propagate to a chain's inputs, so the inputs are prioritized
            # explicitly).
            def emit_wsum_tile(j):
                wtile = wpass_pool.tile([P, DIN], fp32, tag="wp")
                nc.sync.dma_start(wtile[:], wsc_d[j * P:(j + 1) * P, :])
                for s4 in range(4):
                    scw = scrw_pool.tile([P, 512], fp32, tag="scw")
                    nc.scalar.activation(
                        scw[:], wtile[:, s4 * 512:(s4 + 1) * 512], Act.Abs,
                        accum_out=wacc[:, 4 * j + s4:4 * j + s4 + 1])

            ws = small.tile([P, 1], fp32, name="ws")        # written later
            inv_ws = small.tile([P, 1], fp32, name="invws")

            # ---- per-token x tile chain (stats + quantize + transpose)
            def emit_x_tile(i):
                xt = xin_pool.tile([P, DIN], fp32, tag="xin")
                nc.scalar.dma_start(xt[:], x_d[i * P:(i + 1) * P, :])
                scr = scr_pool.tile([P, DIN], fp32, tag="scr")
                # scr = x^2 (discarded); accum -> sumsq
                nc.scalar.activation(scr[:], xt[:], Act.Square,
                                     accum_out=sumsq_t[:, i:i + 1])
                if use_gw:
                    yt = scr_pool.tile([P, DIN], fp32, tag="yt")
                    nc.vector.tensor_tensor(out=yt[:], in0=xt[:], in1=gw_sb[:],
                                            op=Alu.mult)
                    src = yt
                else:
                    src = xt
                nc.vector.tensor_reduce(
                    out=amax_t[:, i:i + 1], in_=src[:], op=Alu.max,
                    axis=mybir.AxisListType.X, apply_absolute_value=True)
                # per-token scalars on [P, 1]
                sq = small.tile([P, 1], fp32, tag="sq")
                nc.scalar.activation(sq[:], sumsq_t[:, i:i + 1], Act.Sqrt,
                                     scale=1.0 / DIN, bias=epsn[:])
                d1 = small.tile([P, 1], fp32, tag="d1")
                nc.vector.tensor_scalar(out=d1[:], in0=amax_t[:, i:i + 1],
                                        scalar1=1.0 / QB, scalar2=None,
                                        op0=Alu.mult)
                rsq = small.tile([P, 1], fp32, tag="rsq")
                nc.vector.reciprocal(rsq[:], sq[:])
                # f1 = d1 + EPS_SCALE*sq ; m = 1/f1
                f1 = small.tile([P, 1], fp32, tag="f1")
                nc.vector.tensor_scalar(out=f1[:], in0=sq[:],
                                        scalar1=EPS_SCALE, scalar2=d1[:],
                                        op0=Alu.mult, op1=Alu.add)
                nc.vector.reciprocal(m_t[:, i:i + 1], f1[:])
                xs0 = small.tile([P, 1], fp32, tag="xs0")
                nc.vector.tensor_tensor(out=xs0[:], in0=d1[:], in1=rsq[:],
                                        op=Alu.mult)
                # alpha = (xs0 + eps) * w_scale.  ws is only written after the
                # AllReduce, so tiles emitted before it store xs1 = xs0 + eps
                # and get alpha in one fix-up op once ws exists.
                if i < N_PRE:
                    nc.vector.tensor_scalar(out=xs1_t[:, i:i + 1], in0=xs0[:],
                                            scalar1=EPS_SCALE, scalar2=None,
                                            op0=Alu.add)
                else:
                    nc.vector.tensor_scalar(out=alpha_t[:, i:i + 1],
                                            in0=xs0[:],
                                            scalar1=EPS_SCALE, scalar2=ws[:],
                                            op0=Alu.add, op1=Alu.mult)
                # quantize in place: xt = round(src*m) + C via magic constant
                nc.vector.tensor_scalar(out=xt[:], in0=src[:],
                                        scalar1=m_t[:, i:i + 1],
                                        scalar2=C_MAGIC,
                                        op0=Alu.mult, op1=Alu.add)
                qt8 = q_pool.tile([P, DIN], bf16, tag="q")
                nc.vector.tensor_scalar(out=qt8[:], in0=xt[:], scalar1=C_MAGIC,
                                        scalar2=None, op0=Alu.subtract)
                # one xbar transpose for the whole tile:
                # qT[d_lo, k, t] = qt8[t, 128k + d_lo]
                qT = qt_pool.tile([P, k_tiles, P], bf16, tag="qT")
                nc.sync.dma_start(out=qT[:], in_=qt8[:], transpose=True)
                return qT

            qT_tiles = [None] * t_tiles

            # emission: the whole scale pass at scheduler priority 0
            with tc.high_priority():
                for j in range(wsc_tiles):
                    emit_wsum_tile(j)
            qT_tiles[0] = emit_x_tile(0)
            qT_tiles[1] = emit_x_tile(1)

            # ---- real AllReduce of per-partition |w| sums.  The pin/pout
            # DMAs live on GPSIMD: they wait on the DVE reduce / the
            # collective, and on the scalar or sync queues that wait would
            # head-block the x-tile loads or the wt stream.
            wpart = small.tile([P, 1], fp32)
            with tc.high_priority():
                nc.vector.tensor_reduce(out=wpart[:], in_=wacc[:], op=Alu.add,
                                        axis=mybir.AxisListType.X)
                nc.gpsimd.dma_start(pin_d[:], wpart[:])
                nc.gpsimd.collective_compute(
                    "AllReduce", Alu.add,
                    replica_groups=[list(range(N_CORES))],
                    ins=[pin_d[:]], outs=[pout_d[:]],
                )
                wsum_all = small.tile([P, 1], fp32)
                nc.gpsimd.dma_start(wsum_all[:], pout_d[:])

            # x tiles 2..N_PRE-1 while the collective runs
            for i in range(2, N_PRE):
                qT_tiles[i] = emit_x_tile(i)

            # cross-partition sum + broadcast via ones matmul
            psum_s = psum_s_pool.tile([P, 512], fp32, tag="pss")
            nc.tensor.matmul(psum_s[:, 0:1], ones_sb[:], wsum_all[:],
                             start=True, stop=True)
            ssum = small.tile([P, 1], fp32)
            nc.vector.tensor_copy(ssum[:], psum_s[:, 0:1])
            nc.vector.tensor_scalar(out=ws[:], in0=ssum[:],
                                    scalar1=1.0 / N_W,
                                    scalar2=EPS_SCALE,
                                    op0=Alu.mult, op1=Alu.add)
            nc.vector.reciprocal(inv_ws[:], ws[:])
            # deferred alpha for the pre-AllReduce tiles
            nc.vector.tensor_scalar(out=alpha_t[:, 0:N_PRE],
                                    in0=xs1_t[:, 0:N_PRE],
                                    scalar1=ws[:], scalar2=None, op0=Alu.mult)

            # ---- wt read (16MB) + quantize -> ternary bf16, oc-major so the
            # first oc chunk of all k tiles completes first.
            for oc in range(oc_chunks):
                osl = slice(oc * oc_sz, (oc + 1) * oc_sz)
                for k in range(k_tiles):
                    st = stage_pool.tile([P, oc_sz], fp32, tag="st")
                    nc.sync.dma_start(st[:], wt_d[k * P:(k + 1) * P, osl])
                    tw1 = tw_pool.tile([P, oc_sz], fp32, tag="tw1")
                    nc.vector.tensor_scalar(out=tw1[:], in0=st[:],
                                            scalar1=inv_ws[:],
                                            scalar2=C_MAGIC,
                                            op0=Alu.mult, op1=Alu.add)
                    tw2 = tw_pool.tile([P, oc_sz], fp32, tag="tw2")
                    # clip in C-space: exact for |q| <= ~2^22
                    nc.vector.tensor_scalar(out=tw2[:], in0=tw1[:],
                                            scalar1=C_MAGIC + 1.0,
                                            scalar2=C_MAGIC - 1.0,
                                            op0=Alu.min, op1=Alu.max)
                    nc.vector.tensor_scalar(out=wq[k][:, osl], in0=tw2[:],
                                            scalar1=C_MAGIC, scalar2=None,
                                            op0=Alu.subtract)

            # ---- matmul chains for one tile: 4 PSUM chunks, ACT drains into
            # one [P, O_SH] staging tile, a single per-tile out DMA on SYNC.
            osb_tiles = {}

            def emit_mm_chain(i, oc, oc_major=False):
                pt = psum_pool.tile([P, oc_sz], fp32, tag="ps")
                qT = qT_tiles[i]
                for k in range(k_tiles):
                    nc.tensor.matmul(pt[:], qT[:, k, :],
                                     wq[k][:, oc * oc_sz:(oc + 1) * oc_sz],
                                     start=(k == 0), stop=(k == k_tiles - 1))
                if oc_major:
                    # phase 1 walks oc-major across tiles: per-chunk staging
                    # + per-chunk out DMA (a per-tile osb would pin N_PH1
                    # buffers at once)
                    osb = outc_pool.tile([P, oc_sz], fp32, tag="oc")
                    nc.scalar.activation(osb[:], pt[:], Act.Copy,
                                         scale=alpha_t[:, i:i + 1])
                    nc.sync.dma_start(
                        out_d[i * P:(i + 1) * P,
                              oc * oc_sz:(oc + 1) * oc_sz], osb[:])
                    return
                if oc == 0:
                    osb_tiles[i] = out_pool.tile([P, O_SH], fp32, tag="o",
                                                 name="osb")
                osb = osb_tiles[i]
                nc.scalar.activation(osb[:, oc * oc_sz:(oc + 1) * oc_sz],
                                     pt[:], Act.Copy,
                                     scale=alpha_t[:, i:i + 1])
                if oc == oc_chunks - 1:
                    nc.sync.dma_start(out_d[i * P:(i + 1) * P, :], osb[:])

            # phase 1: oc-major across tiles 0..N_PH1-1 (paced by quantize).
            # tile-major within the last oc pass so each tile's out DMA can
            # issue as soon as its four chunks are drained.
            for oc in range(oc_chunks):
                for i in range(N_PH1):
                    emit_mm_chain(i, oc, oc_major=True)

            # phase 2: tile-major with x-chain lookahead
            for i in range(N_PRE, N_PH1 + LOOKAHEAD):
                qT_tiles[i] = emit_x_tile(i)
            for i in range(N_PH1, t_tiles):
                j = i + LOOKAHEAD
                if max(N_PRE, N_PH1 + LOOKAHEAD) <= j < t_tiles:
                    qT_tiles[j] = emit_x_tile(j)
                for oc in range(oc_chunks):
                    emit_mm_chain(i, oc)

    nc.compile()
    return nc


# ---------------------------------------------------------------- host wrapper

_CACHED = {}


def _get_nc(use_gw):
    key = ("nc", use_gw)
    if key not in _CACHED:
        _CACHED[key] = build_bass(use_gw=use_gw)
    return _CACHED[key]


def kernel(x: np.ndarray, weight: np.ndarray, norm_weight: np.ndarray) -> np.ndarray:
    from concourse.bass_utils import run_bass_kernel_spmd

    assert x.shape == (B, S, DIN) and weight.shape == (DOUT, DIN)
    use_gw = not bool(np.all(norm_weight == 1.0))
    x_flat = np.ascontiguousarray(x.reshape(TOKENS, DIN), dtype=np.float32)
    w = np.ascontiguousarray(weight, dtype=np.float32)
    wt_full = np.ascontiguousarray(w.T)  # [DIN, DOUT]

    in_maps = []
    for c in range(N_CORES):
        tg, og = divmod(c, OG)
        im = {
            "x": np.ascontiguousarray(x_flat[tg * T_SH:(tg + 1) * T_SH]),
            "wt": np.ascontiguousarray(wt_full[:, og * O_SH:(og + 1) * O_SH]),
            "wsc": np.ascontiguousarray(w[c * WSC_ROWS:(c + 1) * WSC_ROWS]),
        }
        if use_gw:
            im["gw"] = np.ascontiguousarray(
                np.broadcast_to(norm_weight.astype(np.float32), (P, DIN)))
        in_maps.append(im)

    nc = _get_nc(use_gw)
    res = run_bass_kernel_spmd(nc, in_maps, core_ids=list(range(N_CORES)))
    _CACHED["last_results"] = res

    out = np.empty((TOKENS, DOUT), dtype=np.float32)
    for c in range(N_CORES):
        tg, og = divmod(c, OG)
        out[tg * T_SH:(tg + 1) * T_SH, og * O_SH:(og + 1) * O_SH] = \
            res.results[c]["out"]
    return out.reshape(B, S, DOUT)
